# revision 1
# baseline (speedup 1.0000x reference)
"""Trainium2 Bass kernel for nn_LowRankProjection: y = (spikes @ V) @ U.T.

Strategy (data-parallel over batch, 8 cores):
  - Host pre-layouts:
      sT   = spikes.T shard  [N_PRE, B/8]   (contraction dim on partitions)
      Vd   = V rearranged to [128, (N_PRE/128)*R] so lhsT chunks are slices
      Ut   = U.T             [R, N_POST]
      Rm   = 4x stacked I_32 [128, R]       (strip-reduction matmul weight)
  - Device, per core (all matmuls exact fp32):
      phase 1: 4-way col-group packed accumulation over 128 k-chunks:
               z4[32g+r, b] += V_k.T @ sT_k for k % 4 == g  (tile_position)
      reduce:  zT = Rm.T @ z4  (one matmul contracts the 4 strips)
      phase 2: replicate zT and Ut across 4 partition strips, then 4-way
               row-group packed matmuls: y[b_chunk, p] = zT_chunk.T @ Ut_chunk
               -> natural-layout y, so host unshard is a plain concat.
  - Memory-bound: per core 32 MiB in + 32 MiB out + 4 MiB weights.
"""

import numpy as np

import concourse.bacc as bacc
import concourse.mybir as mybir
import concourse.tile as tile
from concourse.bass_utils import run_bass_kernel_spmd

B, N_PRE, N_POST, R = 4096, 16384, 16384, 32
N_CORES = 8
BSH = B // N_CORES  # 512 batch rows per core
P = 128
KC = N_PRE // P  # 128 contraction chunks
F32 = mybir.dt.float32

KPER = 8  # k-chunks per input DMA (2 MiB)
NPC = 8  # 512-wide output chunks per output DMA (2 MiB)


def _body(tc, y, sT, vd, ut, rm):
    nc = tc.nc
    with (
        tc.tile_pool(name="w", bufs=1) as wpool,
        tc.tile_pool(name="s", bufs=3) as spool,
        tc.tile_pool(name="o", bufs=3) as opool,
        tc.tile_pool(name="zps", bufs=1, space="PSUM") as zpspool,
        tc.tile_pool(name="yps", bufs=4, space="PSUM") as ypspool,
    ):
        # Weights go on the gpsimd (SWDGE) queue so they don't serialize
        # ahead of the spikes stream in sync's HWDGE FIFO.
        v_sb = wpool.tile([P, KC * R], F32)
        nc.gpsimd.dma_start(v_sb[:], vd[:])
        rm_sb = wpool.tile([P, R], F32)
        nc.gpsimd.dma_start(rm_sb[:], rm[:])
        # Ut replicated across 4 partition strips: strip 0 from DRAM, rest
        # via SBUF->SBUF DMA (no extra HBM traffic).
        ut4 = wpool.tile([P, N_POST], F32)
        nc.gpsimd.dma_start(ut4[0:R, :], ut[:])
        for g in range(1, 4):
            nc.gpsimd.dma_start(ut4[g * R : (g + 1) * R, :], ut4[0:R, :])

        # Phase 1: z4 [128, BSH] = 4 col-group partial sums over k-chunks.
        z4ps = zpspool.tile([P, BSH], F32, tag="z4")
        for ci in range(KC // KPER):
            s_tile = spool.tile([P, KPER, BSH], F32)
            src = sT[ci * KPER * P : (ci + 1) * KPER * P, :].rearrange(
                "(a p) b -> p a b", p=P
            )
            nc.sync.dma_start(s_tile[:], src)
            for j in range(KPER):
                k = ci * KPER + j
                g = k % 4
                nc.tensor.matmul(
                    z4ps[g * R : (g + 1) * R, :],
                    v_sb[:, k * R : (k + 1) * R],
                    s_tile[:, j, :],
                    start=(k < 4),
                    stop=(k >= KC - 4),
                    tile_position=(0, g * R),
                    # 4 interleaved per-strip groups share one bank; CoreSim's
                    # zero-region tracker is bank-coarse but HW has_written is
                    # per partition row (validated on HW, rel err 2.7e-7).
                    skip_group_check=True,
                )

        # Strip reduction via stacked-identity matmul, then replicate zT
        # into 4 partition strips for phase-2 row-group packing.
        z4_sb = wpool.tile([P, BSH], F32)
        nc.vector.tensor_copy(z4_sb[:], z4ps[:])
        zps2 = zpspool.tile([R, BSH], F32, tag="zred")
        nc.tensor.matmul(zps2[:], rm_sb[:], z4_sb[:], start=True, stop=True)
        zt4 = wpool.tile([P, BSH], F32)
        for g in range(4):
            nc.vector.tensor_copy(zt4[g * R : (g + 1) * R, :], zps2[:])

        # Phase 2: y[b_chunk, :] = zT_chunk.T @ Ut, 4-way row-group packed.
        for bi in range(BSH // P):
            for grp in range(N_POST // (512 * NPC)):
                o_tile = opool.tile([P, NPC * 512], F32)
                for j in range(NPC):
                    n0 = grp * NPC * 512 + j * 512
                    g = j % 4
                    yp = ypspool.tile([P, 512], F32)
                    nc.tensor.matmul(
                        yp[:],
                        zt4[g * R : (g + 1) * R, bi * P : (bi + 1) * P],
                        ut4[g * R : (g + 1) * R, n0 : n0 + 512],
                        start=True,
                        stop=True,
                        tile_position=(g * R, 0),
                    )
                    nc.vector.tensor_copy(o_tile[:, j * 512 : (j + 1) * 512], yp[:])
                # Stores on the scalar-engine HWDGE ring (second physical
                # ring) so they don't share sync's FIFO with input loads.
                nc.scalar.dma_start(
                    y[bi * P : (bi + 1) * P, grp * NPC * 512 : (grp + 1) * NPC * 512],
                    o_tile[:],
                )


_NC_CACHE = None


def _build():
    global _NC_CACHE
    if _NC_CACHE is None:
        nc = bacc.Bacc(
            "TRN2", target_bir_lowering=False, debug=False, num_devices=N_CORES
        )
        sT = nc.dram_tensor("sT", [N_PRE, BSH], F32, kind="ExternalInput").ap()
        vd = nc.dram_tensor("Vd", [P, KC * R], F32, kind="ExternalInput").ap()
        ut = nc.dram_tensor("Ut", [R, N_POST], F32, kind="ExternalInput").ap()
        rm = nc.dram_tensor("Rm", [P, R], F32, kind="ExternalInput").ap()
        y = nc.dram_tensor("y", [BSH, N_POST], F32, kind="ExternalOutput").ap()
        with tile.TileContext(nc) as tc:
            _body(tc, y, sT, vd, ut, rm)
        nc.compile()
        _NC_CACHE = nc
    return _NC_CACHE


def _prep_inputs(spikes, U, V):
    spikes = np.ascontiguousarray(spikes, dtype=np.float32)
    sT = np.ascontiguousarray(spikes.T)  # [N_PRE, B]
    vd = np.ascontiguousarray(
        np.asarray(V, dtype=np.float32)
        .reshape(KC, P, R)
        .transpose(1, 0, 2)
        .reshape(P, KC * R)
    )
    ut = np.ascontiguousarray(np.asarray(U, dtype=np.float32).T)  # [R, N_POST]
    rm = np.tile(np.eye(R, dtype=np.float32), (P // R, 1))  # [P, R]
    in_maps = []
    for c in range(N_CORES):
        in_maps.append(
            {
                "sT": np.ascontiguousarray(sT[:, c * BSH : (c + 1) * BSH]),
                "Vd": vd,
                "Ut": ut,
                "Rm": rm,
            }
        )
    return in_maps


def _run(spikes, U, V, **run_kwargs):
    nc = _build()
    in_maps = _prep_inputs(spikes, U, V)
    res = run_bass_kernel_spmd(nc, in_maps, list(range(N_CORES)), **run_kwargs)
    y = np.concatenate([res.results[c]["y"] for c in range(N_CORES)], axis=0)
    return y, res


def kernel(spikes, U, V, mask_row_ptr=None, mask_col_idx=None, mask_values=None):
    y, _ = _run(spikes, U, V)
    return y



# revision 2
# speedup vs baseline: 1.7070x; 1.7070x over previous
"""Trainium2 Bass kernel for nn_LowRankProjection: y = (spikes @ V) @ U.T.

Strategy (data-parallel over batch, 8 cores), fp16 wire format:
  - Host pre-layouts (all fp16 — harness gate is rel_err < 2e-2, fp16
    costs ~1e-3, and halving the bytes halves the HBM-bound runtime):
      sP  = spikes shard packed to the exact SBUF tile layout
            [NT*128, KPER*BSH] so every input DMA is one fully
            contiguous 1 MiB transfer.
      Vd  = V rearranged to [128, KC*R] so lhsT chunks are slices.
      Ut  = U.T [R, N_POST]; replicated on-device to 4 partition strips.
      Rm  = 4x stacked I_32 [128, R] (strip-reduction matmul weight).
  - Device, per core:
      phase 1: 4-way col-group packed accumulation over 128 k-chunks:
               z4[32g+r, b] += V_k.T @ sT_k for k % 4 == g (tile_position)
      reduce:  zT = Rm.T @ z4 (one matmul contracts the 4 strips)
      phase 2: 4-way row-group packed matmuls into 4-bank PSUM regions;
               PSUM->SBUF casts to fp16 alternate vector/scalar engines;
               stores go out on the scalar HWDGE ring (input loads own
               the sync ring).
  - HBM per core: 16 MiB in + 16 MiB out + ~2 MiB weights ~= 95 us
    roofline at 358 GB/s.
"""

import numpy as np

import concourse.bacc as bacc
import concourse.mybir as mybir
import concourse.tile as tile
from concourse.bass_utils import run_bass_kernel_spmd

B, N_PRE, N_POST, R = 4096, 16384, 16384, 32
N_CORES = 8
BSH = B // N_CORES  # 512 batch rows per core
P = 128
KC = N_PRE // P  # 128 contraction chunks
F16 = mybir.dt.float16
F32 = mybir.dt.float32

KPER = 8  # k-chunks per input DMA tile (1 MiB fp16)
NT = KC // KPER  # 16 input tiles


def _body(tc, y, sP, vd, ut, rm):
    nc = tc.nc
    with (
        tc.tile_pool(name="w", bufs=1) as wpool,
        tc.tile_pool(name="s", bufs=3) as spool,
        tc.tile_pool(name="o", bufs=3) as opool,
    ):
        # V + reduction weights on the gpsimd (SWDGE) queue so they don't
        # serialize ahead of the spikes stream in sync's HWDGE FIFO.
        v_sb = wpool.tile([P, KC * R], F16)
        nc.gpsimd.dma_start(v_sb[:], vd[:])
        rm_sb = wpool.tile([P, R], F16)
        nc.gpsimd.dma_start(rm_sb[:], rm[:])
        # Ut on the scalar HWDGE ring (idle until phase 2's stores);
        # replicate across 4 partition strips via SBUF->SBUF DMA.
        ut4 = wpool.tile([P, N_POST], F16)
        nc.scalar.dma_start(ut4[0:R, :], ut[:])
        for g in range(1, 4):
            nc.scalar.dma_start(ut4[g * R : (g + 1) * R, :], ut4[0:R, :])

        zt4 = wpool.tile([P, BSH], F16)
        # Phase 1: z4 [128, BSH] = 4 col-group partial sums over k-chunks.
        with tc.tile_pool(name="zps", bufs=1, space="PSUM") as zpspool:
            z4ps = zpspool.tile([P, BSH], F32, tag="z4")
            for t in range(NT):
                s_tile = spool.tile([P, KPER * BSH], F16)
                nc.sync.dma_start(s_tile[:], sP[t * P : (t + 1) * P, :])
                for j in range(KPER):
                    k = t * KPER + j
                    g = k % 4
                    nc.tensor.matmul(
                        z4ps[g * R : (g + 1) * R, :],
                        v_sb[:, k * R : (k + 1) * R],
                        s_tile[:, j * BSH : (j + 1) * BSH],
                        start=(k < 4),
                        stop=(k >= KC - 4),
                        tile_position=(0, g * R),
                        # 4 interleaved per-strip groups share one bank;
                        # CoreSim's zero-region tracker is bank-coarse but
                        # HW has_written is per partition row.
                        skip_group_check=True,
                    )

            # Strip reduction via stacked-identity matmul, then replicate
            # zT into 4 partition strips for phase-2 row-group packing.
            z4_sb = wpool.tile([P, BSH], F16)
            nc.vector.tensor_copy(z4_sb[:], z4ps[:])
            zps2 = zpspool.tile([R, BSH], F32, tag="zred")
            nc.tensor.matmul(zps2[:], rm_sb[:], z4_sb[:], start=True, stop=True)
            for g in range(4):
                nc.vector.tensor_copy(zt4[g * R : (g + 1) * R, :], zps2[:])

        # Phase 2: y[b_chunk, :] = zT_chunk.T @ Ut, 4-way row-group packed
        # quartets filling 4-bank PSUM regions, double-buffered.
        with tc.tile_pool(name="yps", bufs=2, space="PSUM") as ypspool:
            cp = 0
            for bi in range(BSH // P):
                for grp in range(N_POST // 4096):
                    o_tile = opool.tile([P, 4096], F16)
                    for h in range(2):
                        yp = ypspool.tile([P, 2048], F32)
                        for u in range(4):
                            jj = grp * 8 + h * 4 + u
                            n0 = jj * 512
                            nc.tensor.matmul(
                                yp[:, u * 512 : (u + 1) * 512],
                                zt4[u * R : (u + 1) * R, bi * P : (bi + 1) * P],
                                ut4[u * R : (u + 1) * R, n0 : n0 + 512],
                                start=True,
                                stop=True,
                                tile_position=(u * R, 0),
                            )
                        dst = o_tile[:, h * 2048 : (h + 1) * 2048]
                        # Split PSUM->SBUF casts across the two engines
                        # with PSUM ports.
                        if cp % 2 == 0:
                            nc.vector.tensor_copy(dst, yp[:])
                        else:
                            nc.scalar.copy(dst, yp[:])
                        cp += 1
                    nc.scalar.dma_start(
                        y[bi * P : (bi + 1) * P, grp * 4096 : (grp + 1) * 4096],
                        o_tile[:],
                    )


_NC_CACHE = None


def _build():
    global _NC_CACHE
    if _NC_CACHE is None:
        nc = bacc.Bacc(
            "TRN2", target_bir_lowering=False, debug=False, num_devices=N_CORES
        )
        sP = nc.dram_tensor("sP", [NT * P, KPER * BSH], F16, kind="ExternalInput").ap()
        vd = nc.dram_tensor("Vd", [P, KC * R], F16, kind="ExternalInput").ap()
        ut = nc.dram_tensor("Ut", [R, N_POST], F16, kind="ExternalInput").ap()
        rm = nc.dram_tensor("Rm", [P, R], F16, kind="ExternalInput").ap()
        y = nc.dram_tensor("y", [BSH, N_POST], F16, kind="ExternalOutput").ap()
        with tile.TileContext(nc) as tc:
            _body(tc, y, sP, vd, ut, rm)
        nc.compile()
        _NC_CACHE = nc
    return _NC_CACHE


def _prep_inputs(spikes, U, V):
    spikes = np.asarray(spikes, dtype=np.float32)
    vd = np.ascontiguousarray(
        np.asarray(V, dtype=np.float32)
        .reshape(KC, P, R)
        .transpose(1, 0, 2)
        .reshape(P, KC * R)
        .astype(np.float16)
    )
    ut = np.ascontiguousarray(np.asarray(U, dtype=np.float32).T.astype(np.float16))
    rm = np.tile(np.eye(R, dtype=np.float16), (P // R, 1))
    in_maps = []
    for c in range(N_CORES):
        # [N_PRE, BSH] shard transpose (cache-friendly per-core blocks),
        # then pack to the SBUF tile layout [t, p, j, b] so each input
        # DMA is one fully contiguous 1 MiB block.
        xt = spikes[c * BSH : (c + 1) * BSH].T.astype(np.float16)
        sp = np.ascontiguousarray(
            xt.reshape(NT, KPER, P, BSH).transpose(0, 2, 1, 3)
        ).reshape(NT * P, KPER * BSH)
        in_maps.append({"sP": sp, "Vd": vd, "Ut": ut, "Rm": rm})
    return in_maps


def _run(spikes, U, V, **run_kwargs):
    nc = _build()
    in_maps = _prep_inputs(spikes, U, V)
    res = run_bass_kernel_spmd(nc, in_maps, list(range(N_CORES)), **run_kwargs)
    y = np.concatenate([res.results[c]["y"] for c in range(N_CORES)], axis=0).astype(
        np.float32
    )
    return y, res


def kernel(spikes, U, V, mask_row_ptr=None, mask_col_idx=None, mask_values=None):
    y, _ = _run(spikes, U, V)
    return y


# revision 3
# speedup vs baseline: 1.7784x; 1.0418x over previous
"""Trainium2 Bass kernel for nn_LowRankProjection: y = (spikes @ V) @ U.T.

Strategy (data-parallel over batch, 8 cores), fp16 wire format:
  - Host pre-layouts (all fp16 — harness gate is rel_err < 2e-2, fp16
    costs ~1e-3, and halving the bytes halves the HBM-bound runtime):
      sP  = spikes shard packed to the exact SBUF tile layout
            [SB*NT*128, KPER*BSB] so every input DMA is one fully
            contiguous 1 MiB transfer.
      Vd  = V rearranged to [128, KC*R] so lhsT chunks are slices.
      Ut  = U.T [R, N_POST]; replicated on-device to 4 partition strips.
      Rm  = 4x stacked I_32 [128, R] (strip-reduction matmul weight).
  - Device, per core, PIPELINED over 4 batch sub-blocks of 128 rows so
    the input stream (sync HWDGE ring) and output stream (scalar HWDGE
    ring) overlap on the shared SDMA engines. Per sub-block:
      project: 4-way col-group packed accumulation over 128 k-chunks:
               z4[32g+r, b] += V_k.T @ sT_k for k % 4 == g (tile_position)
      reduce:  zT = Rm.T @ z4 (one matmul contracts the 4 strips),
               replicated to 4 partition strips for row-group packing
      expand:  4-way row-group packed matmuls into 2-bank PSUM regions;
               PSUM->SBUF casts to fp16 alternate vector/scalar engines;
               stores go out on the scalar ring.
  - HBM per core: 16 MiB in + 16 MiB out + ~2 MiB weights ~= 95 us
    roofline at 358 GB/s.
"""

import numpy as np

import concourse.bacc as bacc
import concourse.mybir as mybir
import concourse.tile as tile
from concourse.bass_utils import run_bass_kernel_spmd

B, N_PRE, N_POST, R = 4096, 16384, 16384, 32
N_CORES = 8
BSH = B // N_CORES  # 512 batch rows per core
P = 128
KC = N_PRE // P  # 128 contraction chunks
F16 = mybir.dt.float16
F32 = mybir.dt.float32

SB = 4  # pipelined batch sub-blocks per core
BSB = BSH // SB  # 128 batch rows per sub-block
KPER = 32  # k-chunks per input DMA tile (1 MiB fp16)
NT = KC // KPER  # 4 input tiles per sub-block


def _body(tc, y, sP, vd, ut, rm):
    nc = tc.nc
    with (
        tc.tile_pool(name="w", bufs=1) as wpool,
        tc.tile_pool(name="s", bufs=3) as spool,
        tc.tile_pool(name="o", bufs=4) as opool,
        tc.tile_pool(name="z", bufs=2) as zpool,
        tc.tile_pool(name="zps", bufs=1, space="PSUM") as zpspool,
        tc.tile_pool(name="yps", bufs=3, space="PSUM") as ypspool,
    ):
        # V + reduction weights on the gpsimd (SWDGE) queue so they don't
        # serialize ahead of the spikes stream in sync's HWDGE FIFO.
        v_sb = wpool.tile([P, KC * R], F16)
        nc.gpsimd.dma_start(v_sb[:], vd[:])
        rm_sb = wpool.tile([P, R], F16)
        nc.gpsimd.dma_start(rm_sb[:], rm[:])
        # Ut on the scalar HWDGE ring (its stores only start ~30us in);
        # replicate across 4 partition strips via SBUF->SBUF DMA.
        ut4 = wpool.tile([P, N_POST], F16)
        nc.scalar.dma_start(ut4[0:R, :], ut[:])
        for g in range(1, 4):
            nc.scalar.dma_start(ut4[g * R : (g + 1) * R, :], ut4[0:R, :])

        cp = 0
        for sb in range(SB):
            # --- project: z4 [128, BSB] = 4 col-group partial sums ---
            z4ps = zpspool.tile([P, BSB], F32, tag="z4")
            for t in range(NT):
                s_tile = spool.tile([P, KPER * BSB], F16)
                nc.sync.dma_start(
                    s_tile[:], sP[(sb * NT + t) * P : (sb * NT + t + 1) * P, :]
                )
                for j in range(KPER):
                    k = t * KPER + j
                    g = k % 4
                    nc.tensor.matmul(
                        z4ps[g * R : (g + 1) * R, :],
                        v_sb[:, k * R : (k + 1) * R],
                        s_tile[:, j * BSB : (j + 1) * BSB],
                        start=(k < 4),
                        stop=(k >= KC - 4),
                        tile_position=(0, g * R),
                        # 4 interleaved per-strip groups share one bank;
                        # CoreSim's zero-region tracker is bank-coarse but
                        # HW has_written is per partition row.
                        skip_group_check=True,
                    )

            # --- reduce strips, replicate zT to 4 partition strips ---
            z4_sb = zpool.tile([P, BSB], F16, tag="z4sb")
            nc.vector.tensor_copy(z4_sb[:], z4ps[:])
            zps2 = zpspool.tile([R, BSB], F32, tag="zred")
            nc.tensor.matmul(zps2[:], rm_sb[:], z4_sb[:], start=True, stop=True)
            zt4 = zpool.tile([P, BSB], F16, tag="zt4")
            for g in range(4):
                nc.vector.tensor_copy(zt4[g * R : (g + 1) * R, :], zps2[:])

            # --- expand: y[sb block, :] = zT.T @ Ut, row-group packed ---
            for grp in range(N_POST // 4096):
                o_tile = opool.tile([P, 4096], F16)
                for h in range(4):
                    yp = ypspool.tile([P, 1024], F32)
                    for u in range(2):
                        jj = grp * 8 + h * 2 + u
                        g = jj % 4
                        n0 = jj * 512
                        nc.tensor.matmul(
                            yp[:, u * 512 : (u + 1) * 512],
                            zt4[g * R : (g + 1) * R, :],
                            ut4[g * R : (g + 1) * R, n0 : n0 + 512],
                            start=True,
                            stop=True,
                            tile_position=(g * R, 0),
                        )
                    dst = o_tile[:, h * 1024 : (h + 1) * 1024]
                    # Split PSUM->SBUF casts across the two engines
                    # with PSUM ports.
                    if cp % 2 == 0:
                        nc.vector.tensor_copy(dst, yp[:])
                    else:
                        nc.scalar.copy(dst, yp[:])
                    cp += 1
                nc.scalar.dma_start(
                    y[sb * P : (sb + 1) * P, grp * 4096 : (grp + 1) * 4096],
                    o_tile[:],
                )


_NC_CACHE = None


def _build():
    global _NC_CACHE
    if _NC_CACHE is None:
        nc = bacc.Bacc(
            "TRN2", target_bir_lowering=False, debug=False, num_devices=N_CORES
        )
        sP = nc.dram_tensor(
            "sP", [SB * NT * P, KPER * BSB], F16, kind="ExternalInput"
        ).ap()
        vd = nc.dram_tensor("Vd", [P, KC * R], F16, kind="ExternalInput").ap()
        ut = nc.dram_tensor("Ut", [R, N_POST], F16, kind="ExternalInput").ap()
        rm = nc.dram_tensor("Rm", [P, R], F16, kind="ExternalInput").ap()
        y = nc.dram_tensor("y", [BSH, N_POST], F16, kind="ExternalOutput").ap()
        with tile.TileContext(nc) as tc:
            _body(tc, y, sP, vd, ut, rm)
        nc.compile()
        _NC_CACHE = nc
    return _NC_CACHE


def _prep_inputs(spikes, U, V):
    spikes = np.asarray(spikes, dtype=np.float32)
    vd = np.ascontiguousarray(
        np.asarray(V, dtype=np.float32)
        .reshape(KC, P, R)
        .transpose(1, 0, 2)
        .reshape(P, KC * R)
        .astype(np.float16)
    )
    ut = np.ascontiguousarray(np.asarray(U, dtype=np.float32).T.astype(np.float16))
    rm = np.tile(np.eye(R, dtype=np.float16), (P // R, 1))
    in_maps = []
    for c in range(N_CORES):
        # [N_PRE, BSH] shard transpose (cache-friendly per-core blocks),
        # then pack to the SBUF tile layout [sb, t, p, j, b] so each
        # input DMA is one fully contiguous 1 MiB block.
        xt = spikes[c * BSH : (c + 1) * BSH].T.astype(np.float16)
        sp = np.ascontiguousarray(
            xt.reshape(NT, KPER, P, SB, BSB).transpose(3, 0, 2, 1, 4)
        ).reshape(SB * NT * P, KPER * BSB)
        in_maps.append({"sP": sp, "Vd": vd, "Ut": ut, "Rm": rm})
    return in_maps


def _run(spikes, U, V, **run_kwargs):
    nc = _build()
    in_maps = _prep_inputs(spikes, U, V)
    res = run_bass_kernel_spmd(nc, in_maps, list(range(N_CORES)), **run_kwargs)
    y = np.concatenate([res.results[c]["y"] for c in range(N_CORES)], axis=0).astype(
        np.float32
    )
    return y, res


def kernel(spikes, U, V, mask_row_ptr=None, mask_col_idx=None, mask_values=None):
    y, _ = _run(spikes, U, V)
    return y


# revision 6
# speedup vs baseline: 1.8103x; 1.0179x over previous
"""Trainium2 Bass kernel for nn_LowRankProjection: y = (spikes @ V) @ U.T.

Strategy (data-parallel over batch, 8 cores), fp16 wire format:
  - Host pre-layouts (all fp16 — harness gate is rel_err < 2e-2, fp16
    costs ~1e-3, and halving the bytes halves the HBM-bound runtime):
      sP  = spikes shard packed to the exact SBUF tile layout
            [SB*NT*128, KPER*BSB] so every input DMA is one fully
            contiguous 1 MiB transfer.
      Vd  = V rearranged to [128, KC*R] so lhsT chunks are slices.
      Ut  = U.T [R, N_POST]; replicated on-device to 4 partition strips.
      Rm  = 4x stacked I_32 [128, R] (strip-reduction matmul weight).
  - Device, per core, PIPELINED over 4 batch sub-blocks of 128 rows so
    the input stream (sync HWDGE ring) and output stream (scalar HWDGE
    ring) overlap on the shared SDMA engines. Per sub-block:
      project: 4-way col-group packed accumulation over 128 k-chunks:
               z4[32g+r, b] += V_k.T @ sT_k for k % 4 == g (tile_position)
      reduce:  zT = Rm.T @ z4 (one matmul contracts the 4 strips),
               replicated to 4 partition strips for row-group packing
      expand:  4-way row-group packed matmuls into 2-bank PSUM regions;
               PSUM->SBUF casts to fp16 alternate vector/scalar engines;
               stores go out on the scalar ring.
  - HBM per core: 16 MiB in + 16 MiB out + ~2 MiB weights ~= 95 us
    roofline at 358 GB/s.
"""

import numpy as np

import concourse.bacc as bacc
import concourse.mybir as mybir
import concourse.tile as tile
from concourse.bass_utils import run_bass_kernel_spmd

B, N_PRE, N_POST, R = 4096, 16384, 16384, 32
N_CORES = 8
BSH = B // N_CORES  # 512 batch rows per core
P = 128
KC = N_PRE // P  # 128 contraction chunks
F16 = mybir.dt.float16
F32 = mybir.dt.float32

SB = 4  # pipelined batch sub-blocks per core
BSB = BSH // SB  # 128 batch rows per sub-block
KPER = 64  # k-chunks per input DMA tile (2 MiB fp16)
NT = KC // KPER  # 2 input tiles per sub-block
OW = 8192  # output tile width (2 MiB fp16 stores)


def _body(tc, y, sP, vd, ut, rm):
    nc = tc.nc
    with (
        tc.tile_pool(name="w", bufs=1) as wpool,
        tc.tile_pool(name="s", bufs=3) as spool,
        tc.tile_pool(name="o", bufs=4) as opool,
        tc.tile_pool(name="z", bufs=2) as zpool,
        tc.tile_pool(name="zps", bufs=1, space="PSUM") as zpspool,
        tc.tile_pool(name="yps", bufs=3, space="PSUM") as ypspool,
    ):
        # ALL weight traffic on the gpsimd (SWDGE) queue: it must not
        # occupy the sync HWDGE ring (spikes stream) nor the scalar ring
        # (stores) — each HWDGE ring drains one DMA at a time, so a slow
        # SBUF->SBUF replication there stalls the whole store pipeline.
        v_sb = wpool.tile([P, KC * R], F16)
        nc.gpsimd.dma_start(v_sb[:], vd[:])
        rm_sb = wpool.tile([P, R], F16)
        nc.gpsimd.dma_start(rm_sb[:], rm[:])
        ut4 = wpool.tile([P, N_POST], F16)
        nc.gpsimd.dma_start(ut4[0:R, :], ut[:])
        for g in range(1, 4):
            nc.gpsimd.dma_start(ut4[g * R : (g + 1) * R, :], ut4[0:R, :])

        cp = 0
        for sb in range(SB):
            # --- project: z4 [128, BSB] = 4 col-group partial sums ---
            z4ps = zpspool.tile([P, BSB], F32, tag="z4")
            for t in range(NT):
                s_tile = spool.tile([P, KPER * BSB], F16)
                nc.sync.dma_start(
                    s_tile[:], sP[(sb * NT + t) * P : (sb * NT + t + 1) * P, :]
                )
                for j in range(KPER):
                    k = t * KPER + j
                    g = k % 4
                    nc.tensor.matmul(
                        z4ps[g * R : (g + 1) * R, :],
                        v_sb[:, k * R : (k + 1) * R],
                        s_tile[:, j * BSB : (j + 1) * BSB],
                        start=(k < 4),
                        stop=(k >= KC - 4),
                        tile_position=(0, g * R),
                        # 4 interleaved per-strip groups share one bank;
                        # CoreSim's zero-region tracker is bank-coarse but
                        # HW has_written is per partition row.
                        skip_group_check=True,
                    )

            # --- reduce strips, replicate zT to 4 partition strips ---
            z4_sb = zpool.tile([P, BSB], F16, tag="z4sb")
            nc.vector.tensor_copy(z4_sb[:], z4ps[:])
            zps2 = zpspool.tile([R, BSB], F32, tag="zred")
            nc.tensor.matmul(zps2[:], rm_sb[:], z4_sb[:], start=True, stop=True)
            zt4 = zpool.tile([P, BSB], F16, tag="zt4")
            for g in range(4):
                nc.vector.tensor_copy(zt4[g * R : (g + 1) * R, :], zps2[:])

            # --- expand: y[sb block, :] = zT.T @ Ut, row-group packed ---
            for grp in range(N_POST // OW):
                o_tile = opool.tile([P, OW], F16)
                for h in range(OW // 1024):
                    yp = ypspool.tile([P, 1024], F32)
                    for u in range(2):
                        jj = (grp * OW + h * 1024 + u * 512) // 512
                        g = jj % 4
                        n0 = jj * 512
                        nc.tensor.matmul(
                            yp[:, u * 512 : (u + 1) * 512],
                            zt4[g * R : (g + 1) * R, :],
                            ut4[g * R : (g + 1) * R, n0 : n0 + 512],
                            start=True,
                            stop=True,
                            tile_position=(g * R, 0),
                        )
                    dst = o_tile[:, h * 1024 : (h + 1) * 1024]
                    # Split PSUM->SBUF casts across the two engines
                    # with PSUM ports.
                    if cp % 2 == 0:
                        nc.vector.tensor_copy(dst, yp[:])
                    else:
                        nc.scalar.copy(dst, yp[:])
                    cp += 1
                nc.scalar.dma_start(
                    y[sb * P : (sb + 1) * P, grp * OW : (grp + 1) * OW],
                    o_tile[:],
                )


_NC_CACHE = None


def _build():
    global _NC_CACHE
    if _NC_CACHE is None:
        nc = bacc.Bacc(
            "TRN2", target_bir_lowering=False, debug=False, num_devices=N_CORES
        )
        sP = nc.dram_tensor(
            "sP", [SB * NT * P, KPER * BSB], F16, kind="ExternalInput"
        ).ap()
        vd = nc.dram_tensor("Vd", [P, KC * R], F16, kind="ExternalInput").ap()
        ut = nc.dram_tensor("Ut", [R, N_POST], F16, kind="ExternalInput").ap()
        rm = nc.dram_tensor("Rm", [P, R], F16, kind="ExternalInput").ap()
        y = nc.dram_tensor("y", [BSH, N_POST], F16, kind="ExternalOutput").ap()
        with tile.TileContext(nc) as tc:
            _body(tc, y, sP, vd, ut, rm)
        nc.compile()
        _NC_CACHE = nc
    return _NC_CACHE


def _prep_inputs(spikes, U, V):
    spikes = np.asarray(spikes, dtype=np.float32)
    vd = np.ascontiguousarray(
        np.asarray(V, dtype=np.float32)
        .reshape(KC, P, R)
        .transpose(1, 0, 2)
        .reshape(P, KC * R)
        .astype(np.float16)
    )
    ut = np.ascontiguousarray(np.asarray(U, dtype=np.float32).T.astype(np.float16))
    rm = np.tile(np.eye(R, dtype=np.float16), (P // R, 1))
    in_maps = []
    for c in range(N_CORES):
        # [N_PRE, BSH] shard transpose (cache-friendly per-core blocks),
        # then pack to the SBUF tile layout [sb, t, p, j, b] so each
        # input DMA is one fully contiguous 1 MiB block.
        xt = spikes[c * BSH : (c + 1) * BSH].T.astype(np.float16)
        sp = np.ascontiguousarray(
            xt.reshape(NT, KPER, P, SB, BSB).transpose(3, 0, 2, 1, 4)
        ).reshape(SB * NT * P, KPER * BSB)
        in_maps.append({"sP": sp, "Vd": vd, "Ut": ut, "Rm": rm})
    return in_maps


def _run(spikes, U, V, **run_kwargs):
    nc = _build()
    in_maps = _prep_inputs(spikes, U, V)
    res = run_bass_kernel_spmd(nc, in_maps, list(range(N_CORES)), **run_kwargs)
    y = np.concatenate([res.results[c]["y"] for c in range(N_CORES)], axis=0).astype(
        np.float32
    )
    return y, res


def kernel(spikes, U, V, mask_row_ptr=None, mask_col_idx=None, mask_values=None):
    y, _ = _run(spikes, U, V)
    return y


# revision 10
# speedup vs baseline: 2.0419x; 1.1280x over previous
"""Trainium2 Bass kernel for nn_LowRankProjection: y = (spikes @ V) @ U.T.

Strategy (data-parallel over batch, 8 cores), fp16 wire format:
  - Host pre-layouts (all fp16 — harness gate is rel_err < 2e-2, fp16
    costs ~1e-3, and halving the bytes halves the HBM-bound runtime):
      sP  = spikes shard packed to the exact SBUF tile layout
            [SB*NT*128, KPER*BSB] so every input DMA is one fully
            contiguous 1 MiB transfer.
      Vd  = V rearranged to [128, KC*R] so lhsT chunks are slices.
      Ut  = U.T [R, N_POST]; replicated on-device to 4 partition strips.
      Rm  = 4x stacked I_32 [128, R] (strip-reduction matmul weight).
  - Device, per core, PIPELINED over 4 batch sub-blocks of 128 rows so
    the input stream (sync HWDGE ring) and output stream (scalar HWDGE
    ring) overlap on the shared SDMA engines. Per sub-block:
      project: 4-way col-group packed accumulation over 128 k-chunks:
               z4[32g+r, b] += V_k.T @ sT_k for k % 4 == g (tile_position)
      reduce:  zT = Rm.T @ z4 (one matmul contracts the 4 strips),
               replicated to 4 partition strips for row-group packing
      expand:  4-way row-group packed matmuls into 2-bank PSUM regions;
               PSUM->SBUF casts to fp16 alternate vector/scalar engines;
               stores go out on the scalar ring.
  - HBM per core: 16 MiB in + 16 MiB out + ~2 MiB weights ~= 95 us
    roofline at 358 GB/s.
"""

import numpy as np

import concourse.bacc as bacc
import concourse.mybir as mybir
import concourse.tile as tile
from concourse.bass_utils import run_bass_kernel_spmd

B, N_PRE, N_POST, R = 4096, 16384, 16384, 32
N_CORES = 8
BSH = B // N_CORES  # 512 batch rows per core
P = 128
KC = N_PRE // P  # 128 contraction chunks
F16 = mybir.dt.float16
F32 = mybir.dt.float32

SB = 4  # pipelined batch sub-blocks per core
BSB = BSH // SB  # 128 batch rows per sub-block
KPER = 64  # k-chunks per input DMA tile (2 MiB fp16)
NT = KC // KPER  # 2 input tiles per sub-block
OW = 8192  # output tile width (2 MiB fp16 stores)


def _body(tc, y, sP, vd, ut, rm):
    nc = tc.nc
    with (
        tc.tile_pool(name="w", bufs=1) as wpool,
        tc.tile_pool(name="s", bufs=3) as spool,
        tc.tile_pool(name="o", bufs=4) as opool,
        tc.tile_pool(name="z", bufs=2) as zpool,
        tc.tile_pool(name="zps", bufs=1, space="PSUM") as zpspool,
        tc.tile_pool(name="yps", bufs=3, space="PSUM") as ypspool,
    ):
        # V + reduction weights on the gpsimd (SWDGE) queue so they don't
        # serialize ahead of the spikes stream in sync's HWDGE FIFO.
        v_sb = wpool.tile([P, KC * R], F16)
        nc.gpsimd.dma_start(v_sb[:], vd[:])
        rm_sb = wpool.tile([P, R], F16)
        nc.gpsimd.dma_start(rm_sb[:], rm[:])
        # Ut is COLUMN-PARTITIONED across the 4 row-group strips (strip g
        # holds only the n-chunks with chunk%4 == g), so no on-device
        # replication is needed at all — one 1 MiB load on the scalar
        # ring (idle until the first store ~30us in).
        utp = wpool.tile([P, N_POST // 4], F16)
        nc.scalar.dma_start(utp[:], ut[:])

        cp = 0
        for sb in range(SB):
            # --- project: z4 [128, BSB] = 4 col-group partial sums ---
            z4ps = zpspool.tile([P, BSB], F32, tag="z4")
            for t in range(NT):
                s_tile = spool.tile([P, KPER * BSB], F16)
                nc.sync.dma_start(
                    s_tile[:], sP[(sb * NT + t) * P : (sb * NT + t + 1) * P, :]
                )
                for j in range(KPER):
                    k = t * KPER + j
                    g = k % 4
                    nc.tensor.matmul(
                        z4ps[g * R : (g + 1) * R, :],
                        v_sb[:, k * R : (k + 1) * R],
                        s_tile[:, j * BSB : (j + 1) * BSB],
                        start=(k < 4),
                        stop=(k >= KC - 4),
                        tile_position=(0, g * R),
                        # 4 interleaved per-strip groups share one bank;
                        # CoreSim's zero-region tracker is bank-coarse but
                        # HW has_written is per partition row.
                        skip_group_check=True,
                    )

            # --- reduce strips, replicate zT to 4 partition strips ---
            z4_sb = zpool.tile([P, BSB], F16, tag="z4sb")
            nc.vector.tensor_copy(z4_sb[:], z4ps[:])
            zps2 = zpspool.tile([R, BSB], F32, tag="zred")
            nc.tensor.matmul(zps2[:], rm_sb[:], z4_sb[:], start=True, stop=True)
            zt4 = zpool.tile([P, BSB], F16, tag="zt4")
            for g in range(4):
                nc.vector.tensor_copy(zt4[g * R : (g + 1) * R, :], zps2[:])

            # --- expand: y[sb block, :] = zT.T @ Ut, row-group packed.
            # Group g computes the n-chunks with chunk%4 == g from its
            # own column-partitioned slice of utp.
            for grp in range(N_POST // OW):
                o_tile = opool.tile([P, OW], F16)
                for h in range(OW // 1024):
                    yp = ypspool.tile([P, 1024], F32)
                    for u in range(2):
                        g = (h % 2) * 2 + u
                        c = grp * (OW // 2048) + h // 2
                        nc.tensor.matmul(
                            yp[:, u * 512 : (u + 1) * 512],
                            zt4[g * R : (g + 1) * R, :],
                            utp[g * R : (g + 1) * R, c * 512 : (c + 1) * 512],
                            start=True,
                            stop=True,
                            tile_position=(g * R, 0),
                        )
                    dst = o_tile[:, h * 1024 : (h + 1) * 1024]
                    # Split PSUM->SBUF casts across the two engines
                    # with PSUM ports.
                    if cp % 2 == 0:
                        nc.vector.tensor_copy(dst, yp[:])
                    else:
                        nc.scalar.copy(dst, yp[:])
                    cp += 1
                nc.scalar.dma_start(
                    y[sb * P : (sb + 1) * P, grp * OW : (grp + 1) * OW],
                    o_tile[:],
                )


_NC_CACHE = None


def _build():
    global _NC_CACHE
    if _NC_CACHE is None:
        nc = bacc.Bacc(
            "TRN2", target_bir_lowering=False, debug=False, num_devices=N_CORES
        )
        sP = nc.dram_tensor(
            "sP", [SB * NT * P, KPER * BSB], F16, kind="ExternalInput"
        ).ap()
        vd = nc.dram_tensor("Vd", [P, KC * R], F16, kind="ExternalInput").ap()
        ut = nc.dram_tensor("Ut", [P, N_POST // 4], F16, kind="ExternalInput").ap()
        rm = nc.dram_tensor("Rm", [P, R], F16, kind="ExternalInput").ap()
        y = nc.dram_tensor("y", [BSH, N_POST], F16, kind="ExternalOutput").ap()
        with tile.TileContext(nc) as tc:
            _body(tc, y, sP, vd, ut, rm)
        nc.compile()
        _NC_CACHE = nc
    return _NC_CACHE


def _prep_inputs(spikes, U, V):
    spikes = np.asarray(spikes, dtype=np.float32)
    vd = np.ascontiguousarray(
        np.asarray(V, dtype=np.float32)
        .reshape(KC, P, R)
        .transpose(1, 0, 2)
        .reshape(P, KC * R)
        .astype(np.float16)
    )
    # Column-partitioned Ut: utp[32g+r, c*512+s] = U.T[r, c*2048+g*512+s],
    # so row-group strip g holds exactly the n-chunks it computes.
    ut = np.ascontiguousarray(
        np.asarray(U, dtype=np.float32)
        .T.astype(np.float16)
        .reshape(R, N_POST // 2048, 4, 512)
        .transpose(2, 0, 1, 3)
        .reshape(P, N_POST // 4)
    )
    rm = np.tile(np.eye(R, dtype=np.float16), (P // R, 1))
    in_maps = []
    for c in range(N_CORES):
        # [N_PRE, BSH] shard transpose (cache-friendly per-core blocks),
        # then pack to the SBUF tile layout [sb, t, p, j, b] so each
        # input DMA is one fully contiguous 1 MiB block.
        xt = spikes[c * BSH : (c + 1) * BSH].T.astype(np.float16)
        sp = np.ascontiguousarray(
            xt.reshape(NT, KPER, P, SB, BSB).transpose(3, 0, 2, 1, 4)
        ).reshape(SB * NT * P, KPER * BSB)
        in_maps.append({"sP": sp, "Vd": vd, "Ut": ut, "Rm": rm})
    return in_maps


def _run(spikes, U, V, **run_kwargs):
    nc = _build()
    in_maps = _prep_inputs(spikes, U, V)
    res = run_bass_kernel_spmd(nc, in_maps, list(range(N_CORES)), **run_kwargs)
    y = np.concatenate([res.results[c]["y"] for c in range(N_CORES)], axis=0).astype(
        np.float32
    )
    return y, res


def kernel(spikes, U, V, mask_row_ptr=None, mask_col_idx=None, mask_values=None):
    y, _ = _run(spikes, U, V)
    return y


# revision 13
# speedup vs baseline: 2.4328x; 1.1915x over previous
"""Trainium2 Bass kernel for nn_LowRankProjection: y = (spikes @ V) @ U.T.

Strategy (data-parallel over batch, 8 cores), fp16 wire format:
  - Host pre-layouts (all fp16 — harness gate is rel_err < 2e-2, fp16
    costs ~1e-3, and halving the bytes halves the HBM-bound runtime):
      sP  = spikes shard packed to the exact SBUF tile layout
            [SB*NT*128, KPER*BSB] so every input DMA is one fully
            contiguous 1 MiB transfer.
      Vd  = V rearranged to [128, KC*R] so lhsT chunks are slices.
      Ut  = U.T [R, N_POST]; replicated on-device to 4 partition strips.
      Rm  = 4x stacked I_32 [128, R] (strip-reduction matmul weight).
  - Device, per core, PIPELINED over 4 batch sub-blocks of 128 rows so
    the input stream (sync HWDGE ring) and output stream (scalar HWDGE
    ring) overlap on the shared SDMA engines. Per sub-block:
      project: 4-way col-group packed accumulation over 128 k-chunks:
               z4[32g+r, b] += V_k.T @ sT_k for k % 4 == g (tile_position)
      reduce:  zT = Rm.T @ z4 (one matmul contracts the 4 strips),
               replicated to 4 partition strips for row-group packing
      expand:  4-way row-group packed matmuls into 2-bank PSUM regions;
               PSUM->SBUF casts to fp16 alternate vector/scalar engines;
               stores go out on the scalar ring.
  - HBM per core: 16 MiB in + 16 MiB out + ~2 MiB weights ~= 95 us
    roofline at 358 GB/s.
"""

import numpy as np

import concourse.bacc as bacc
import concourse.mybir as mybir
import concourse.tile as tile
from concourse.bass_utils import run_bass_kernel_spmd

B, N_PRE, N_POST, R = 4096, 16384, 16384, 32
N_CORES = 8
BSH = B // N_CORES  # 512 batch rows per core
P = 128
KC = N_PRE // P  # 128 contraction chunks
F16 = mybir.dt.float16
F32 = mybir.dt.float32

SB = 4  # pipelined batch sub-blocks per core
BSB = BSH // SB  # 128 batch rows per sub-block
KPER = 64  # k-chunks per input DMA tile (2 MiB fp16)
NT = KC // KPER  # 2 input tiles per sub-block
OW = 8192  # output tile width (2 MiB fp16 stores)


def _body(tc, y, sP, vd, ut, rm):
    nc = tc.nc
    with (
        tc.tile_pool(name="w", bufs=1) as wpool,
        tc.tile_pool(name="s", bufs=3) as spool,
        tc.tile_pool(name="o", bufs=4) as opool,
        tc.tile_pool(name="z", bufs=2) as zpool,
        tc.tile_pool(name="zps", bufs=1, space="PSUM") as zpspool,
        tc.tile_pool(name="yps", bufs=3, space="PSUM") as ypspool,
    ):
        # V + reduction weights at the head of the sync HWDGE ring: they
        # drain in ~5us and the first project matmuls need them; on the
        # slower SWDGE path they'd gate the whole PE stream ~20us.
        v_sb = wpool.tile([P, KC * R], F16)
        nc.sync.dma_start(v_sb[:], vd[:])
        rm_sb = wpool.tile([P, R], F16)
        nc.sync.dma_start(rm_sb[:], rm[:])
        # Ut is COLUMN-PARTITIONED across the 4 row-group strips (strip g
        # holds only the n-chunks with chunk%4 == g), so no on-device
        # replication is needed at all — one 1 MiB load on the scalar
        # ring (idle until the first store ~30us in).
        utp = wpool.tile([P, N_POST // 4], F16)
        nc.scalar.dma_start(utp[:], ut[:])

        cp = 0
        for sb in range(SB):
            # --- project: z4 [128, BSB] = 4 col-group partial sums ---
            z4ps = zpspool.tile([P, BSB], F32, tag="z4")
            for t in range(NT):
                s_tile = spool.tile([P, KPER * BSB], F16)
                # Alternate input tiles between the sync HWDGE ring and
                # the gpsimd SWDGE queue: each ring drains one DMA at a
                # time (~260 GB/s), so two queues are needed to reach the
                # 358 GB/s HBM ceiling.
                eng = nc.sync if (sb * NT + t) % 2 == 0 else nc.gpsimd
                eng.dma_start(
                    s_tile[:], sP[(sb * NT + t) * P : (sb * NT + t + 1) * P, :]
                )
                for j in range(KPER):
                    k = t * KPER + j
                    g = k % 4
                    nc.tensor.matmul(
                        z4ps[g * R : (g + 1) * R, :],
                        v_sb[:, k * R : (k + 1) * R],
                        s_tile[:, j * BSB : (j + 1) * BSB],
                        start=(k < 4),
                        stop=(k >= KC - 4),
                        tile_position=(0, g * R),
                        # 4 interleaved per-strip groups share one bank;
                        # CoreSim's zero-region tracker is bank-coarse but
                        # HW has_written is per partition row.
                        skip_group_check=True,
                    )

            # --- reduce strips, replicate zT to 4 partition strips ---
            z4_sb = zpool.tile([P, BSB], F16, tag="z4sb")
            nc.vector.tensor_copy(z4_sb[:], z4ps[:])
            zps2 = zpspool.tile([R, BSB], F32, tag="zred")
            nc.tensor.matmul(zps2[:], rm_sb[:], z4_sb[:], start=True, stop=True)
            zt4 = zpool.tile([P, BSB], F16, tag="zt4")
            for g in range(4):
                nc.vector.tensor_copy(zt4[g * R : (g + 1) * R, :], zps2[:])

            # --- expand: y[sb block, :] = zT.T @ Ut, row-group packed.
            # Group g computes the n-chunks with chunk%4 == g from its
            # own column-partitioned slice of utp.
            for grp in range(N_POST // OW):
                o_tile = opool.tile([P, OW], F16)
                for h in range(OW // 1024):
                    yp = ypspool.tile([P, 1024], F32)
                    for u in range(2):
                        g = (h % 2) * 2 + u
                        c = grp * (OW // 2048) + h // 2
                        nc.tensor.matmul(
                            yp[:, u * 512 : (u + 1) * 512],
                            zt4[g * R : (g + 1) * R, :],
                            utp[g * R : (g + 1) * R, c * 512 : (c + 1) * 512],
                            start=True,
                            stop=True,
                            tile_position=(g * R, 0),
                        )
                    dst = o_tile[:, h * 1024 : (h + 1) * 1024]
                    # Split PSUM->SBUF casts across the two engines
                    # with PSUM ports.
                    if cp % 2 == 0:
                        nc.vector.tensor_copy(dst, yp[:])
                    else:
                        nc.scalar.copy(dst, yp[:])
                    cp += 1
                # Alternate stores between the scalar HWDGE ring and the
                # gpsimd SWDGE queue for the same ring-cap reason.
                oeng = nc.scalar if (sb * (N_POST // OW) + grp) % 2 == 0 else nc.gpsimd
                oeng.dma_start(
                    y[sb * P : (sb + 1) * P, grp * OW : (grp + 1) * OW],
                    o_tile[:],
                )


_NC_CACHE = None


def _build():
    global _NC_CACHE
    if _NC_CACHE is None:
        nc = bacc.Bacc(
            "TRN2", target_bir_lowering=False, debug=False, num_devices=N_CORES
        )
        sP = nc.dram_tensor(
            "sP", [SB * NT * P, KPER * BSB], F16, kind="ExternalInput"
        ).ap()
        vd = nc.dram_tensor("Vd", [P, KC * R], F16, kind="ExternalInput").ap()
        ut = nc.dram_tensor("Ut", [P, N_POST // 4], F16, kind="ExternalInput").ap()
        rm = nc.dram_tensor("Rm", [P, R], F16, kind="ExternalInput").ap()
        y = nc.dram_tensor("y", [BSH, N_POST], F16, kind="ExternalOutput").ap()
        with tile.TileContext(nc) as tc:
            _body(tc, y, sP, vd, ut, rm)
        nc.compile()
        _NC_CACHE = nc
    return _NC_CACHE


def _prep_inputs(spikes, U, V):
    spikes = np.asarray(spikes, dtype=np.float32)
    vd = np.ascontiguousarray(
        np.asarray(V, dtype=np.float32)
        .reshape(KC, P, R)
        .transpose(1, 0, 2)
        .reshape(P, KC * R)
        .astype(np.float16)
    )
    # Column-partitioned Ut: utp[32g+r, c*512+s] = U.T[r, c*2048+g*512+s],
    # so row-group strip g holds exactly the n-chunks it computes.
    ut = np.ascontiguousarray(
        np.asarray(U, dtype=np.float32)
        .T.astype(np.float16)
        .reshape(R, N_POST // 2048, 4, 512)
        .transpose(2, 0, 1, 3)
        .reshape(P, N_POST // 4)
    )
    rm = np.tile(np.eye(R, dtype=np.float16), (P // R, 1))
    in_maps = []
    for c in range(N_CORES):
        # [N_PRE, BSH] shard transpose (cache-friendly per-core blocks),
        # then pack to the SBUF tile layout [sb, t, p, j, b] so each
        # input DMA is one fully contiguous 1 MiB block.
        xt = spikes[c * BSH : (c + 1) * BSH].T.astype(np.float16)
        sp = np.ascontiguousarray(
            xt.reshape(NT, KPER, P, SB, BSB).transpose(3, 0, 2, 1, 4)
        ).reshape(SB * NT * P, KPER * BSB)
        in_maps.append({"sP": sp, "Vd": vd, "Ut": ut, "Rm": rm})
    return in_maps


def _run(spikes, U, V, **run_kwargs):
    nc = _build()
    in_maps = _prep_inputs(spikes, U, V)
    res = run_bass_kernel_spmd(nc, in_maps, list(range(N_CORES)), **run_kwargs)
    y = np.concatenate([res.results[c]["y"] for c in range(N_CORES)], axis=0).astype(
        np.float32
    )
    return y, res


def kernel(spikes, U, V, mask_row_ptr=None, mask_col_idx=None, mask_values=None):
    y, _ = _run(spikes, U, V)
    return y


# revision 15
# speedup vs baseline: 2.4771x; 1.0182x over previous
"""Trainium2 Bass kernel for nn_LowRankProjection: y = (spikes @ V) @ U.T.

Strategy (data-parallel over batch, 8 cores), fp16 wire format:
  - Host pre-layouts (all fp16 — harness gate is rel_err < 2e-2, fp16
    costs ~1e-3, and halving the bytes halves the HBM-bound runtime):
      sP  = spikes shard packed to the exact SBUF tile layout
            [SB*NT*128, KPER*BSB] so every input DMA is one fully
            contiguous 1 MiB transfer.
      Vd  = V rearranged to [128, KC*R] so lhsT chunks are slices.
      Ut  = U.T [R, N_POST]; replicated on-device to 4 partition strips.
      Rm  = 4x stacked I_32 [128, R] (strip-reduction matmul weight).
  - Device, per core, PIPELINED over 4 batch sub-blocks of 128 rows so
    the input stream (sync HWDGE ring) and output stream (scalar HWDGE
    ring) overlap on the shared SDMA engines. Per sub-block:
      project: 4-way col-group packed accumulation over 128 k-chunks:
               z4[32g+r, b] += V_k.T @ sT_k for k % 4 == g (tile_position)
      reduce:  zT = Rm.T @ z4 (one matmul contracts the 4 strips),
               replicated to 4 partition strips for row-group packing
      expand:  4-way row-group packed matmuls into 2-bank PSUM regions;
               PSUM->SBUF casts to fp16 alternate vector/scalar engines;
               stores go out on the scalar ring.
  - HBM per core: 16 MiB in + 16 MiB out + ~2 MiB weights ~= 95 us
    roofline at 358 GB/s.
"""

import numpy as np

import concourse.bacc as bacc
import concourse.mybir as mybir
import concourse.tile as tile
from concourse.bass_utils import run_bass_kernel_spmd

B, N_PRE, N_POST, R = 4096, 16384, 16384, 32
N_CORES = 8
BSH = B // N_CORES  # 512 batch rows per core
P = 128
KC = N_PRE // P  # 128 contraction chunks
F16 = mybir.dt.float16
F32 = mybir.dt.float32

SB = 4  # pipelined batch sub-blocks per core
BSB = BSH // SB  # 128 batch rows per sub-block
KPER = 64  # k-chunks per input DMA tile (2 MiB fp16)
NT = KC // KPER  # 2 input tiles per sub-block
OW = 8192  # output tile width (2 MiB fp16 stores)


def _body(tc, y, sP, vd, ut, rm):
    nc = tc.nc
    with (
        tc.tile_pool(name="w", bufs=1) as wpool,
        tc.tile_pool(name="s", bufs=3) as spool,
        tc.tile_pool(name="o", bufs=6) as opool,
        tc.tile_pool(name="z", bufs=2) as zpool,
        tc.tile_pool(name="zps", bufs=1, space="PSUM") as zpspool,
        tc.tile_pool(name="yps", bufs=3, space="PSUM") as ypspool,
    ):
        # V + reduction weights at the head of the sync HWDGE ring: they
        # drain in ~5us and the first project matmuls need them; on the
        # slower SWDGE path they'd gate the whole PE stream ~20us.
        v_sb = wpool.tile([P, KC * R], F16)
        nc.sync.dma_start(v_sb[:], vd[:])
        rm_sb = wpool.tile([P, R], F16)
        nc.sync.dma_start(rm_sb[:], rm[:])
        # Ut is COLUMN-PARTITIONED across the 4 row-group strips (strip g
        # holds only the n-chunks with chunk%4 == g), so no on-device
        # replication is needed at all — one 1 MiB load on the scalar
        # ring (idle until the first store ~30us in).
        utp = wpool.tile([P, N_POST // 4], F16)
        nc.scalar.dma_start(utp[:], ut[:])

        cp = 0
        for sb in range(SB):
            # --- project: z4 [128, BSB] = 4 col-group partial sums ---
            z4ps = zpspool.tile([P, BSB], F32, tag="z4")
            for t in range(NT):
                s_tile = spool.tile([P, KPER * BSB], F16)
                # Alternate input tiles between the sync HWDGE ring and
                # the gpsimd SWDGE queue: each ring drains one DMA at a
                # time (~260 GB/s), so two queues are needed to reach the
                # 358 GB/s HBM ceiling.
                eng = nc.sync if (sb * NT + t) % 2 == 0 else nc.gpsimd
                eng.dma_start(
                    s_tile[:], sP[(sb * NT + t) * P : (sb * NT + t + 1) * P, :]
                )
                for j in range(KPER):
                    k = t * KPER + j
                    g = k % 4
                    nc.tensor.matmul(
                        z4ps[g * R : (g + 1) * R, :],
                        v_sb[:, k * R : (k + 1) * R],
                        s_tile[:, j * BSB : (j + 1) * BSB],
                        start=(k < 4),
                        stop=(k >= KC - 4),
                        tile_position=(0, g * R),
                        # 4 interleaved per-strip groups share one bank;
                        # CoreSim's zero-region tracker is bank-coarse but
                        # HW has_written is per partition row.
                        skip_group_check=True,
                    )

            # --- reduce strips, replicate zT to 4 partition strips ---
            z4_sb = zpool.tile([P, BSB], F16, tag="z4sb")
            nc.vector.tensor_copy(z4_sb[:], z4ps[:])
            zps2 = zpspool.tile([R, BSB], F32, tag="zred")
            nc.tensor.matmul(zps2[:], rm_sb[:], z4_sb[:], start=True, stop=True)
            zt4 = zpool.tile([P, BSB], F16, tag="zt4")
            for g in range(4):
                nc.vector.tensor_copy(zt4[g * R : (g + 1) * R, :], zps2[:])

            # --- expand: y[sb block, :] = zT.T @ Ut, row-group packed.
            # Group g computes the n-chunks with chunk%4 == g from its
            # own column-partitioned slice of utp.
            for grp in range(N_POST // OW):
                o_tile = opool.tile([P, OW], F16)
                for h in range(OW // 1024):
                    yp = ypspool.tile([P, 1024], F32)
                    for u in range(2):
                        g = (h % 2) * 2 + u
                        c = grp * (OW // 2048) + h // 2
                        nc.tensor.matmul(
                            yp[:, u * 512 : (u + 1) * 512],
                            zt4[g * R : (g + 1) * R, :],
                            utp[g * R : (g + 1) * R, c * 512 : (c + 1) * 512],
                            start=True,
                            stop=True,
                            tile_position=(g * R, 0),
                        )
                    dst = o_tile[:, h * 1024 : (h + 1) * 1024]
                    # Split PSUM->SBUF casts across the two engines
                    # with PSUM ports.
                    if cp % 2 == 0:
                        nc.vector.tensor_copy(dst, yp[:])
                    else:
                        nc.scalar.copy(dst, yp[:])
                    cp += 1
                # Alternate stores between the scalar HWDGE ring and the
                # gpsimd SWDGE queue for the same ring-cap reason. The
                # last sub-block's first store rides the sync ring, which
                # is idle once the input stream finishes.
                if sb == SB - 1 and grp == 0:
                    oeng = nc.sync
                elif (sb * (N_POST // OW) + grp) % 2 == 0:
                    oeng = nc.scalar
                else:
                    oeng = nc.gpsimd
                oeng.dma_start(
                    y[sb * P : (sb + 1) * P, grp * OW : (grp + 1) * OW],
                    o_tile[:],
                )


_NC_CACHE = None


def _build():
    global _NC_CACHE
    if _NC_CACHE is None:
        nc = bacc.Bacc(
            "TRN2", target_bir_lowering=False, debug=False, num_devices=N_CORES
        )
        sP = nc.dram_tensor(
            "sP", [SB * NT * P, KPER * BSB], F16, kind="ExternalInput"
        ).ap()
        vd = nc.dram_tensor("Vd", [P, KC * R], F16, kind="ExternalInput").ap()
        ut = nc.dram_tensor("Ut", [P, N_POST // 4], F16, kind="ExternalInput").ap()
        rm = nc.dram_tensor("Rm", [P, R], F16, kind="ExternalInput").ap()
        y = nc.dram_tensor("y", [BSH, N_POST], F16, kind="ExternalOutput").ap()
        with tile.TileContext(nc) as tc:
            _body(tc, y, sP, vd, ut, rm)
        nc.compile()
        _NC_CACHE = nc
    return _NC_CACHE


def _prep_inputs(spikes, U, V):
    spikes = np.asarray(spikes, dtype=np.float32)
    vd = np.ascontiguousarray(
        np.asarray(V, dtype=np.float32)
        .reshape(KC, P, R)
        .transpose(1, 0, 2)
        .reshape(P, KC * R)
        .astype(np.float16)
    )
    # Column-partitioned Ut: utp[32g+r, c*512+s] = U.T[r, c*2048+g*512+s],
    # so row-group strip g holds exactly the n-chunks it computes.
    ut = np.ascontiguousarray(
        np.asarray(U, dtype=np.float32)
        .T.astype(np.float16)
        .reshape(R, N_POST // 2048, 4, 512)
        .transpose(2, 0, 1, 3)
        .reshape(P, N_POST // 4)
    )
    rm = np.tile(np.eye(R, dtype=np.float16), (P // R, 1))
    in_maps = []
    for c in range(N_CORES):
        # [N_PRE, BSH] shard transpose (cache-friendly per-core blocks),
        # then pack to the SBUF tile layout [sb, t, p, j, b] so each
        # input DMA is one fully contiguous 1 MiB block.
        xt = spikes[c * BSH : (c + 1) * BSH].T.astype(np.float16)
        sp = np.ascontiguousarray(
            xt.reshape(NT, KPER, P, SB, BSB).transpose(3, 0, 2, 1, 4)
        ).reshape(SB * NT * P, KPER * BSB)
        in_maps.append({"sP": sp, "Vd": vd, "Ut": ut, "Rm": rm})
    return in_maps


def _run(spikes, U, V, **run_kwargs):
    nc = _build()
    in_maps = _prep_inputs(spikes, U, V)
    res = run_bass_kernel_spmd(nc, in_maps, list(range(N_CORES)), **run_kwargs)
    y = np.concatenate([res.results[c]["y"] for c in range(N_CORES)], axis=0).astype(
        np.float32
    )
    return y, res


def kernel(spikes, U, V, mask_row_ptr=None, mask_col_idx=None, mask_values=None):
    y, _ = _run(spikes, U, V)
    return y


# revision 18
# speedup vs baseline: 2.5166x; 1.0160x over previous
"""Trainium2 Bass kernel for nn_LowRankProjection: y = (spikes @ V) @ U.T.

Strategy (data-parallel over batch, 8 cores), fp16 wire format:
  - Host pre-layouts (all fp16 — harness gate is rel_err < 2e-2, fp16
    costs ~1e-3, and halving the bytes halves the HBM-bound runtime):
      sP  = spikes shard packed to the exact SBUF tile layout
            [SB*NT*128, KPER*BSB] so every input DMA is one fully
            contiguous 1 MiB transfer.
      Vd  = V rearranged to [128, KC*R] so lhsT chunks are slices.
      Ut  = U.T [R, N_POST]; replicated on-device to 4 partition strips.
      Rm  = 4x stacked I_32 [128, R] (strip-reduction matmul weight).
  - Device, per core, PIPELINED over 4 batch sub-blocks of 128 rows so
    the input stream (sync HWDGE ring) and output stream (scalar HWDGE
    ring) overlap on the shared SDMA engines. Per sub-block:
      project: 4-way col-group packed accumulation over 128 k-chunks:
               z4[32g+r, b] += V_k.T @ sT_k for k % 4 == g (tile_position)
      reduce:  zT = Rm.T @ z4 (one matmul contracts the 4 strips),
               replicated to 4 partition strips for row-group packing
      expand:  4-way row-group packed matmuls into 2-bank PSUM regions;
               PSUM->SBUF casts to fp16 alternate vector/scalar engines;
               stores go out on the scalar ring.
  - HBM per core: 16 MiB in + 16 MiB out + ~2 MiB weights ~= 95 us
    roofline at 358 GB/s.
"""

import numpy as np

import concourse.bacc as bacc
import concourse.mybir as mybir
import concourse.tile as tile
from concourse.bass_utils import run_bass_kernel_spmd

B, N_PRE, N_POST, R = 4096, 16384, 16384, 32
N_CORES = 8
BSH = B // N_CORES  # 512 batch rows per core
P = 128
KC = N_PRE // P  # 128 contraction chunks
F16 = mybir.dt.float16
F32 = mybir.dt.float32

SB = 4  # pipelined batch sub-blocks per core
BSB = BSH // SB  # 128 batch rows per sub-block
KPER = 64  # k-chunks per input DMA tile (2 MiB fp16)
NT = KC // KPER  # 2 input tiles per sub-block
OW = 8192  # output tile width (2 MiB fp16 stores)


def _body(tc, y, sP, vd, ut, rm):
    nc = tc.nc
    with (
        tc.tile_pool(name="w", bufs=1) as wpool,
        tc.tile_pool(name="s", bufs=4) as spool,
        tc.tile_pool(name="o", bufs=6) as opool,
        tc.tile_pool(name="z", bufs=2) as zpool,
        tc.tile_pool(name="zps", bufs=1, space="PSUM") as zpspool,
        tc.tile_pool(name="yps", bufs=3, space="PSUM") as ypspool,
    ):
        # V + reduction weights at the head of the sync HWDGE ring: they
        # drain in ~5us and the first project matmuls need them; on the
        # slower SWDGE path they'd gate the whole PE stream ~20us.
        v_sb = wpool.tile([P, KC * R], F16)
        nc.sync.dma_start(v_sb[:], vd[:])
        rm_sb = wpool.tile([P, R], F16)
        nc.sync.dma_start(rm_sb[:], rm[:])
        # Ut is COLUMN-PARTITIONED across the 4 row-group strips (strip g
        # holds only the n-chunks with chunk%4 == g), so no on-device
        # replication is needed at all — one 1 MiB load on the scalar
        # ring (idle until the first store ~30us in).
        utp = wpool.tile([P, N_POST // 4], F16)
        nc.scalar.dma_start(utp[:], ut[:])

        # ALL input DMAs are emitted up front, alternating between the
        # sync HWDGE ring and the gpsimd SWDGE queue (each ring drains
        # one DMA at a time at ~260 GB/s; two queues reach the 358 GB/s
        # HBM ceiling). Hoisting matters for the SWDGE queue: the Q7
        # emits descriptors in program order, so an input emission
        # placed after a store emission would wait on that store's
        # o_tile production, serializing the input stream behind the
        # expand pipeline.
        s_tiles = []
        for idx in range(SB * NT):
            s_tile = spool.tile([P, KPER * BSB], F16, name="s_tile", tag="s_tile")
            eng = nc.sync if idx % 2 == 0 else nc.gpsimd
            eng.dma_start(s_tile[:], sP[idx * P : (idx + 1) * P, :])
            s_tiles.append(s_tile)

        cp = 0
        for sb in range(SB):
            # --- project: z4 [128, BSB] = 4 col-group partial sums ---
            z4ps = zpspool.tile([P, BSB], F32, tag="z4")
            for t in range(NT):
                s_tile = s_tiles[sb * NT + t]
                for j in range(KPER):
                    k = t * KPER + j
                    g = k % 4
                    nc.tensor.matmul(
                        z4ps[g * R : (g + 1) * R, :],
                        v_sb[:, k * R : (k + 1) * R],
                        s_tile[:, j * BSB : (j + 1) * BSB],
                        start=(k < 4),
                        stop=(k >= KC - 4),
                        tile_position=(0, g * R),
                        # 4 interleaved per-strip groups share one bank;
                        # CoreSim's zero-region tracker is bank-coarse but
                        # HW has_written is per partition row.
                        skip_group_check=True,
                    )

            # --- reduce strips, replicate zT to 4 partition strips ---
            z4_sb = zpool.tile([P, BSB], F16, tag="z4sb")
            nc.vector.tensor_copy(z4_sb[:], z4ps[:])
            zps2 = zpspool.tile([R, BSB], F32, tag="zred")
            nc.tensor.matmul(zps2[:], rm_sb[:], z4_sb[:], start=True, stop=True)
            zt4 = zpool.tile([P, BSB], F16, tag="zt4")
            for g in range(4):
                nc.vector.tensor_copy(zt4[g * R : (g + 1) * R, :], zps2[:])

            # --- expand: y[sb block, :] = zT.T @ Ut, row-group packed.
            # Group g computes the n-chunks with chunk%4 == g from its
            # own column-partitioned slice of utp.
            for grp in range(N_POST // OW):
                o_tile = opool.tile([P, OW], F16)
                for h in range(OW // 1024):
                    yp = ypspool.tile([P, 1024], F32)
                    for u in range(2):
                        g = (h % 2) * 2 + u
                        c = grp * (OW // 2048) + h // 2
                        nc.tensor.matmul(
                            yp[:, u * 512 : (u + 1) * 512],
                            zt4[g * R : (g + 1) * R, :],
                            utp[g * R : (g + 1) * R, c * 512 : (c + 1) * 512],
                            start=True,
                            stop=True,
                            tile_position=(g * R, 0),
                        )
                    dst = o_tile[:, h * 1024 : (h + 1) * 1024]
                    # Split PSUM->SBUF casts across the two engines
                    # with PSUM ports.
                    if cp % 2 == 0:
                        nc.vector.tensor_copy(dst, yp[:])
                    else:
                        nc.scalar.copy(dst, yp[:])
                    cp += 1
                # Alternate stores between the scalar HWDGE ring and the
                # gpsimd SWDGE queue for the same ring-cap reason. The
                # last sub-block's first store rides the sync ring, which
                # is idle once the input stream finishes.
                if sb == SB - 1 and grp == 0:
                    oeng = nc.sync
                elif (sb * (N_POST // OW) + grp) % 2 == 0:
                    oeng = nc.scalar
                else:
                    oeng = nc.gpsimd
                oeng.dma_start(
                    y[sb * P : (sb + 1) * P, grp * OW : (grp + 1) * OW],
                    o_tile[:],
                )


_NC_CACHE = None


def _build():
    global _NC_CACHE
    if _NC_CACHE is None:
        nc = bacc.Bacc(
            "TRN2", target_bir_lowering=False, debug=False, num_devices=N_CORES
        )
        sP = nc.dram_tensor(
            "sP", [SB * NT * P, KPER * BSB], F16, kind="ExternalInput"
        ).ap()
        vd = nc.dram_tensor("Vd", [P, KC * R], F16, kind="ExternalInput").ap()
        ut = nc.dram_tensor("Ut", [P, N_POST // 4], F16, kind="ExternalInput").ap()
        rm = nc.dram_tensor("Rm", [P, R], F16, kind="ExternalInput").ap()
        y = nc.dram_tensor("y", [BSH, N_POST], F16, kind="ExternalOutput").ap()
        with tile.TileContext(nc) as tc:
            _body(tc, y, sP, vd, ut, rm)
        nc.compile()
        _NC_CACHE = nc
    return _NC_CACHE


def _prep_inputs(spikes, U, V):
    spikes = np.asarray(spikes, dtype=np.float32)
    vd = np.ascontiguousarray(
        np.asarray(V, dtype=np.float32)
        .reshape(KC, P, R)
        .transpose(1, 0, 2)
        .reshape(P, KC * R)
        .astype(np.float16)
    )
    # Column-partitioned Ut: utp[32g+r, c*512+s] = U.T[r, c*2048+g*512+s],
    # so row-group strip g holds exactly the n-chunks it computes.
    ut = np.ascontiguousarray(
        np.asarray(U, dtype=np.float32)
        .T.astype(np.float16)
        .reshape(R, N_POST // 2048, 4, 512)
        .transpose(2, 0, 1, 3)
        .reshape(P, N_POST // 4)
    )
    rm = np.tile(np.eye(R, dtype=np.float16), (P // R, 1))
    in_maps = []
    for c in range(N_CORES):
        # [N_PRE, BSH] shard transpose (cache-friendly per-core blocks),
        # then pack to the SBUF tile layout [sb, t, p, j, b] so each
        # input DMA is one fully contiguous 1 MiB block.
        xt = spikes[c * BSH : (c + 1) * BSH].T.astype(np.float16)
        sp = np.ascontiguousarray(
            xt.reshape(NT, KPER, P, SB, BSB).transpose(3, 0, 2, 1, 4)
        ).reshape(SB * NT * P, KPER * BSB)
        in_maps.append({"sP": sp, "Vd": vd, "Ut": ut, "Rm": rm})
    return in_maps


def _run(spikes, U, V, **run_kwargs):
    nc = _build()
    in_maps = _prep_inputs(spikes, U, V)
    res = run_bass_kernel_spmd(nc, in_maps, list(range(N_CORES)), **run_kwargs)
    y = np.concatenate([res.results[c]["y"] for c in range(N_CORES)], axis=0).astype(
        np.float32
    )
    return y, res


def kernel(spikes, U, V, mask_row_ptr=None, mask_col_idx=None, mask_values=None):
    y, _ = _run(spikes, U, V)
    return y
